# revision 2
# baseline (speedup 1.0000x reference)
"""Trainium2 Bass kernel for nn_MessageGcn (GNN message passing).

out = relu( segsum_{recv}(x[send] @ W_f) + segsum_{send}(x[recv] @ W_b)
            + (x @ W_s) * dropout_mask )

Strategy (8 NeuronCores, SPMD, one shared program):
  - Algebraic reorder: aggregate raw x rows per destination FIRST, then apply
    the [128,128] weights once per destination node:
        out[n] = relu( accF[n]@W_f + accB[n]@W_b + (x[n]@W_s)*mask[n] )
    where accF[n] = sum_{e: recv[e]=n} x[send[e]],
          accB[n] = sum_{e: send[e]=n} x[recv[e]].
    This cuts GEMM work 6x vs edge-space GEMMs.
  - Shard destination nodes across 8 cores (12500 nodes each); x is
    replicated so each core gathers source rows locally.
  - Host routing: each edge contributes (src=send, dst=recv, type=F) and
    (src=recv, dst=send, type=B). Contributions are bucketed by
    (core, dst_tile_of_128) and padded to chunks of 128. All cores share one
    compiled program, so per-(type,tile) chunk counts are the max over cores;
    padding rows point at a zeros row of the table with local-dst -1.
  - Device: indirect DMA gathers 128 source rows per chunk into SBUF;
    TensorE computes acc^T[tile] via one-hot matmul
    (acc^T = gathered^T-free: out[feat, dst] = sum_p g[p,feat]*onehot[p,dst])
    accumulating chunks in PSUM; the one-hot is built on VectorE by
    comparing local-dst codes against an iota row. Self-loop rows are
    streamed sequentially and folded in with an identity matrix.
    Then per destination tile: W_f/W_b GEMMs (+ masked W_s GEMM), dropout
    mask from drop_u on VectorE, relu on ScalarE, DMA out (transposed).
"""

import numpy as np

import concourse.bass as bass
import concourse.bacc as bacc
import concourse.mybir as mybir
import concourse.tile as tile
from concourse.bass_utils import run_bass_kernel_spmd
from concourse.masks import make_identity

N = 100000
E = 600000
D = 128
P = 128
NCORES = 8
SHARD = N // NCORES          # 12500 dst nodes per core
TILES = (SHARD + P - 1) // P  # 98 dst tiles per core
SHARD_PAD = TILES * P         # 12544
KEEP_PROB = 0.8
ZROW = N                      # index of the appended zeros row in the table


def _route(senders, receivers):
    """Build per-core gather/onehot metadata. Returns (sched, gidx, ldst):
    sched[t] = (nf, nb) chunks for tile t (shared across cores);
    gidx[c]  = int32 [P, NCH] source-row index per (chunk, slot);
    ldst[c]  = float32 [P, NCH] local dst (0..127) or -1 for padding.
    Chunk order: tile 0 (F chunks.. B chunks), tile 1 (...), ...
    """
    s = senders.astype(np.int64)
    r = receivers.astype(np.int64)
    # contributions: type F: (src=s, dst=r); type B: (src=r, dst=s)
    src = np.concatenate([s, r]).astype(np.int32)
    dst = np.concatenate([r, s]).astype(np.int32)
    typ = np.concatenate([np.zeros(E, np.int8), np.ones(E, np.int8)])

    core = dst // SHARD
    ldst_all = dst - core * SHARD
    tile_id = ldst_all // P
    lcol = (ldst_all % P).astype(np.float32)

    # group key: (core, tile, type)
    key = (core.astype(np.int64) * TILES + tile_id) * 2 + typ
    ngroups = NCORES * TILES * 2
    counts = np.bincount(key, minlength=ngroups).reshape(NCORES, TILES, 2)
    chunks = -(-counts // P)  # ceil
    sched_ft = chunks.max(axis=0)  # [TILES, 2] shared schedule

    # chunk-slot base offset of each (core, tile, type) group in the stream
    per_tile = sched_ft.sum(axis=1)          # chunks per tile
    tile_base = np.concatenate([[0], np.cumsum(per_tile)[:-1]])  # chunk idx
    nch = int(per_tile.sum())
    # group slot base in "slot" units (slot = chunk*P + partition)
    grp_base = np.empty((TILES, 2), np.int64)
    grp_base[:, 0] = tile_base * P
    grp_base[:, 1] = (tile_base + sched_ft[:, 0]) * P

    order = np.argsort(key, kind="stable")
    key_sorted = key[order]
    # rank within group
    grp_start_pos = np.concatenate([[0], np.cumsum(np.bincount(key_sorted, minlength=ngroups))[:-1]])
    rank = np.arange(src.size) - grp_start_pos[key_sorted]

    core_s = core[order]
    tile_s = tile_id[order]
    typ_s = typ[order].astype(np.int64)
    slot = grp_base[tile_s, typ_s] + rank

    gidx = np.full((NCORES, P, nch), ZROW, np.int32)
    ldst = np.full((NCORES, P, nch), -1.0, np.float32)
    gidx[core_s, slot % P, slot // P] = src[order]
    ldst[core_s, slot % P, slot // P] = lcol[order]
    return sched_ft, gidx, ldst, nch


def _build(sched_ft, nch):
    nc = bacc.Bacc(None, target_bir_lowering=False, dynamic_dma_scratch_size=65536)
    xt = nc.dram_tensor("xt", [N + 1, D], mybir.dt.float32, kind="ExternalInput")
    wf = nc.dram_tensor("wf", [D, D], mybir.dt.float32, kind="ExternalInput")
    wb = nc.dram_tensor("wb", [D, D], mybir.dt.float32, kind="ExternalInput")
    ws = nc.dram_tensor("ws", [D, D], mybir.dt.float32, kind="ExternalInput")
    dut = nc.dram_tensor("dut", [P, SHARD_PAD], mybir.dt.float32, kind="ExternalInput")
    gidx = nc.dram_tensor("gidx", [P, nch], mybir.dt.int32, kind="ExternalInput")
    ldst = nc.dram_tensor("ldst", [P, nch], mybir.dt.float32, kind="ExternalInput")
    xown = nc.dram_tensor("xown", [SHARD_PAD, D], mybir.dt.float32, kind="ExternalInput")
    outT = nc.dram_tensor("outT", [P, SHARD_PAD], mybir.dt.float32, kind="ExternalOutput")

    with tile.TileContext(nc) as tc:
        with (
            tc.tile_pool(name="cst", bufs=1) as cst,
            tc.tile_pool(name="stage", bufs=8) as stage,
            tc.tile_pool(name="ohp", bufs=8) as ohp,
            tc.tile_pool(name="accp", bufs=3) as accp,
            tc.tile_pool(name="outp", bufs=3) as outp,
            tc.tile_pool(name="psA", bufs=3, space="PSUM") as psA,
            tc.tile_pool(name="psB", bufs=3, space="PSUM") as psB,
        ):
            iota = cst.tile([P, P], mybir.dt.float32)
            nc.gpsimd.iota(iota[:], [[1, P]], channel_multiplier=0,
                           allow_small_or_imprecise_dtypes=True)
            ident = cst.tile([P, P], mybir.dt.float32)
            make_identity(nc, ident[:])
            wf_t = cst.tile([P, D], mybir.dt.float32)
            nc.sync.dma_start(out=wf_t[:], in_=wf[:])
            wb_t = cst.tile([P, D], mybir.dt.float32)
            nc.sync.dma_start(out=wb_t[:], in_=wb[:])
            ws_t = cst.tile([P, D], mybir.dt.float32)
            nc.sync.dma_start(out=ws_t[:], in_=ws[:])
            gidx_t = cst.tile([P, nch], mybir.dt.int32)
            nc.sync.dma_start(out=gidx_t[:], in_=gidx[:])
            ldst_t = cst.tile([P, nch], mybir.dt.float32)
            nc.sync.dma_start(out=ldst_t[:], in_=ldst[:])

            ci = 0
            for t in range(TILES):
                nf, nb = int(sched_ft[t, 0]), int(sched_ft[t, 1])
                seg = psA.tile([P, 512], mybir.dt.float32, tag="seg")
                # F chunks -> seg[:, 0:128]; B chunks -> seg[:, 128:256];
                # self -> seg[:, 256:384]
                for typi, ntyp in ((0, nf), (1, nb)):
                    for k in range(ntyp):
                        g = stage.tile([P, D], mybir.dt.float32, tag="g")
                        nc.gpsimd.indirect_dma_start(
                            out=g[:], out_offset=None, in_=xt[:],
                            in_offset=bass.IndirectOffsetOnAxis(
                                ap=gidx_t[:, ci:ci + 1], axis=0),
                        )
                        oh = ohp.tile([P, P], mybir.dt.float32, tag="oh")
                        nc.vector.tensor_tensor(
                            out=oh[:],
                            in0=ldst_t[:, ci:ci + 1].to_broadcast([P, P]),
                            in1=iota[:],
                            op=mybir.AluOpType.is_equal,
                        )
                        nc.tensor.matmul(
                            out=seg[:, typi * P:(typi + 1) * P],
                            lhsT=g[:], rhs=oh[:],
                            start=(k == 0), stop=(k == ntyp - 1),
                        )
                        ci += 1
                # self rows: sequential stream + identity matmul
                gs = stage.tile([P, D], mybir.dt.float32, tag="g")
                nc.sync.dma_start(out=gs[:], in_=xown[t * P:(t + 1) * P, :])
                nc.tensor.matmul(out=seg[:, 2 * P:3 * P], lhsT=gs[:],
                                 rhs=ident[:], start=True, stop=True)

                accT = accp.tile([P, 3 * P], mybir.dt.float32, tag="accT")
                nc.scalar.copy(out=accT[:], in_=seg[:, 0:3 * P])

                gem = psB.tile([P, 256], mybir.dt.float32, tag="gem")
                nc.tensor.matmul(out=gem[:, 0:P], lhsT=wf_t[:],
                                 rhs=accT[:, 0:P], start=True, stop=False)
                nc.tensor.matmul(out=gem[:, 0:P], lhsT=wb_t[:],
                                 rhs=accT[:, P:2 * P], start=False, stop=True)
                nc.tensor.matmul(out=gem[:, P:2 * P], lhsT=ws_t[:],
                                 rhs=accT[:, 2 * P:3 * P], start=True, stop=True)

                du = outp.tile([P, P], mybir.dt.float32, tag="du")
                nc.sync.dma_start(out=du[:], in_=dut[:, t * P:(t + 1) * P])
                m = outp.tile([P, P], mybir.dt.float32, tag="m")
                nc.vector.tensor_scalar(
                    out=m[:], in0=du[:], scalar1=KEEP_PROB,
                    scalar2=1.0 / KEEP_PROB,
                    op0=mybir.AluOpType.is_lt, op1=mybir.AluOpType.mult,
                )
                sm = outp.tile([P, P], mybir.dt.float32, tag="sm")
                nc.vector.tensor_tensor(out=sm[:], in0=gem[:, P:2 * P],
                                        in1=m[:], op=mybir.AluOpType.mult)
                tot = outp.tile([P, P], mybir.dt.float32, tag="tot")
                nc.vector.tensor_tensor(out=tot[:], in0=gem[:, 0:P],
                                        in1=sm[:], op=mybir.AluOpType.add)
                ot = outp.tile([P, P], mybir.dt.float32, tag="ot")
                nc.scalar.activation(out=ot[:], in_=tot[:],
                                     func=mybir.ActivationFunctionType.Relu)
                nc.sync.dma_start(out=outT[:, t * P:(t + 1) * P], in_=ot[:])
    nc.compile()
    return nc


def prepare(inputs):
    x = np.asarray(inputs["x"], np.float32)
    W_f = np.asarray(inputs["W_f"], np.float32)
    W_b = np.asarray(inputs["W_b"], np.float32)
    W_s = np.asarray(inputs["W_s"], np.float32)
    drop_u = np.asarray(inputs["drop_u"], np.float32)
    senders = np.asarray(inputs["senders"])
    receivers = np.asarray(inputs["receivers"])

    sched_ft, gidx, ldst, nch = _route(senders, receivers)
    nc = _build(sched_ft, nch)

    xt = np.concatenate([x, np.zeros((1, D), np.float32)], axis=0)
    in_maps = []
    for c in range(NCORES):
        lo = c * SHARD
        du = np.zeros((SHARD_PAD, D), np.float32)
        du[:SHARD] = drop_u[lo:lo + SHARD]
        xo = np.zeros((SHARD_PAD, D), np.float32)
        xo[:SHARD] = x[lo:lo + SHARD]
        in_maps.append({
            "xt": xt, "wf": W_f, "wb": W_b, "ws": W_s,
            "dut": np.ascontiguousarray(du.T),
            "gidx": np.ascontiguousarray(gidx[c]),
            "ldst": np.ascontiguousarray(ldst[c]),
            "xown": xo,
        })
    return nc, in_maps


def kernel(x, W_f, W_b, W_s, drop_u, senders, receivers):
    nc, in_maps = prepare(dict(x=x, W_f=W_f, W_b=W_b, W_s=W_s,
                               drop_u=drop_u, senders=senders,
                               receivers=receivers))
    res = run_bass_kernel_spmd(nc, in_maps, core_ids=list(range(NCORES)))
    out = np.empty((N, D), np.float32)
    for c in range(NCORES):
        out[c * SHARD:(c + 1) * SHARD] = res.results[c]["outT"][:, :SHARD].T
    return out



# revision 3
# speedup vs baseline: 17.5082x; 17.5082x over previous
"""Trainium2 Bass kernel for nn_MessageGcn (GNN message passing).

out = relu( segsum_{recv}(x[send] @ W_f) + segsum_{send}(x[recv] @ W_b)
            + (x @ W_s) * dropout_mask )

Strategy (8 NeuronCores, SPMD, one shared program):
  - Algebraic reorder: aggregate raw x rows per destination FIRST, then apply
    the [128,128] weights once per destination node:
        out[n] = relu( accF[n]@W_f + accB[n]@W_b + (x[n]@W_s)*mask[n] )
    where accF[n] = sum_{e: recv[e]=n} x[send[e]],
          accB[n] = sum_{e: send[e]=n} x[recv[e]].
  - Shard destination nodes across 8 cores (12500 each); x replicated (bf16)
    so each core gathers source rows locally.
  - Host routing: contributions bucketed by (core, dst_tile_of_128, type),
    padded to chunks of 128 (shared schedule = max chunks over cores; padding
    rows point at a zeros row with local-dst -1).
  - Device per dst tile: ONE wide indirect DMA gathers all (nf+nb) chunks of
    128 bf16 source rows into SBUF (amortizes the ~1us SWDGE per-op cost that
    bottlenecked the per-chunk version); per chunk a one-hot built on VectorE
    (ldst code vs iota row) routes rows to dst columns via TensorE matmul
    accumulating acc^T[feat, dst] in PSUM. Self-loop comes in pre-transposed
    (xsT) so W_s GEMM consumes it directly; dropout mask from drop_u on
    VectorE, relu on ScalarE, DMA out transposed.
"""

import numpy as np

import concourse.bass as bass
import concourse.bacc as bacc
import concourse.mybir as mybir
import concourse.tile as tile
from concourse.bass_utils import run_bass_kernel_spmd
from concourse.masks import make_identity

N = 100000
E = 600000
D = 128
P = 128
NCORES = 8
SHARD = N // NCORES          # 12500 dst nodes per core
TILES = (SHARD + P - 1) // P  # 98 dst tiles per core
SHARD_PAD = TILES * P         # 12544
KEEP_PROB = 0.8
ZROW = N                      # index of the appended zeros row in the table

try:
    import ml_dtypes
    BF16 = ml_dtypes.bfloat16
except ImportError:  # fallback: float32 view tricks not needed normally
    BF16 = None


def _to_bf16(a):
    return np.asarray(a, np.float32).astype(BF16)


def _route(senders, receivers):
    """Build per-core gather/onehot metadata. Returns (sched, gidx, ldst, nch):
    sched[t] = (nf, nb) chunks for tile t (shared across cores);
    gidx[c]  = int32 [P, NCH] source-row index per (chunk, slot);
    ldst[c]  = float32 [P, NCH] local dst (0..127) or -1 for padding.
    Chunk order: tile 0 (F chunks.. B chunks), tile 1 (...), ...
    Slot s of a group -> (partition s%128, chunk s//128).
    """
    s = senders.astype(np.int64)
    r = receivers.astype(np.int64)
    src = np.concatenate([s, r]).astype(np.int32)
    dst = np.concatenate([r, s]).astype(np.int32)
    typ = np.concatenate([np.zeros(E, np.int8), np.ones(E, np.int8)])

    core = dst // SHARD
    ldst_all = dst - core * SHARD
    tile_id = ldst_all // P
    lcol = (ldst_all % P).astype(np.float32)

    key = (core.astype(np.int64) * TILES + tile_id) * 2 + typ
    ngroups = NCORES * TILES * 2
    counts = np.bincount(key, minlength=ngroups).reshape(NCORES, TILES, 2)
    chunks = -(-counts // P)  # ceil
    sched_ft = chunks.max(axis=0)  # [TILES, 2] shared schedule

    per_tile = sched_ft.sum(axis=1)
    tile_base = np.concatenate([[0], np.cumsum(per_tile)[:-1]])
    nch = int(per_tile.sum())
    grp_base = np.empty((TILES, 2), np.int64)
    grp_base[:, 0] = tile_base * P
    grp_base[:, 1] = (tile_base + sched_ft[:, 0]) * P

    order = np.argsort(key, kind="stable")
    key_sorted = key[order]
    grp_start_pos = np.concatenate(
        [[0], np.cumsum(np.bincount(key_sorted, minlength=ngroups))[:-1]])
    rank = np.arange(src.size) - grp_start_pos[key_sorted]

    core_s = core[order]
    tile_s = tile_id[order]
    typ_s = typ[order].astype(np.int64)
    slot = grp_base[tile_s, typ_s] + rank

    gidx = np.full((NCORES, P, nch), ZROW, np.int32)
    ldst = np.full((NCORES, P, nch), -1.0, np.float32)
    gidx[core_s, slot % P, slot // P] = src[order]
    ldst[core_s, slot % P, slot // P] = lcol[order]
    return sched_ft, gidx, ldst, nch


def _build(sched_ft, nch):
    nc = bacc.Bacc(None, target_bir_lowering=False,
                   dynamic_dma_scratch_size=65536)
    bf = mybir.dt.bfloat16
    f32 = mybir.dt.float32
    xt = nc.dram_tensor("xt", [N + 1, D], bf, kind="ExternalInput")
    wf = nc.dram_tensor("wf", [D, D], bf, kind="ExternalInput")
    wb = nc.dram_tensor("wb", [D, D], bf, kind="ExternalInput")
    ws = nc.dram_tensor("ws", [D, D], bf, kind="ExternalInput")
    dut = nc.dram_tensor("dut", [P, SHARD_PAD], f32, kind="ExternalInput")
    gidx = nc.dram_tensor("gidx", [P, nch], mybir.dt.int32,
                          kind="ExternalInput")
    ldst = nc.dram_tensor("ldst", [P, nch], bf, kind="ExternalInput")
    xst = nc.dram_tensor("xst", [P, SHARD_PAD], bf, kind="ExternalInput")
    outT = nc.dram_tensor("outT", [P, SHARD_PAD], f32, kind="ExternalOutput")

    with tile.TileContext(nc) as tc:
        with (
            tc.tile_pool(name="cst", bufs=1) as cst,
            tc.tile_pool(name="stage", bufs=4) as stage,
            tc.tile_pool(name="ohp", bufs=8) as ohp,
            tc.tile_pool(name="accp", bufs=3) as accp,
            tc.tile_pool(name="outp", bufs=3) as outp,
            tc.tile_pool(name="psA", bufs=4, space="PSUM") as psA,
            tc.tile_pool(name="psB", bufs=4, space="PSUM") as psB,
        ):
            iota32 = cst.tile([P, P], f32)
            nc.gpsimd.iota(iota32[:], [[1, P]], channel_multiplier=0,
                           allow_small_or_imprecise_dtypes=True)
            iota = cst.tile([P, P], bf)
            nc.scalar.copy(out=iota[:], in_=iota32[:])
            wf_t = cst.tile([P, D], bf)
            nc.sync.dma_start(out=wf_t[:], in_=wf[:])
            wb_t = cst.tile([P, D], bf)
            nc.sync.dma_start(out=wb_t[:], in_=wb[:])
            ws_t = cst.tile([P, D], bf)
            nc.sync.dma_start(out=ws_t[:], in_=ws[:])
            gidx_t = cst.tile([P, nch], mybir.dt.int32)
            nc.sync.dma_start(out=gidx_t[:], in_=gidx[:])
            ldst_t = cst.tile([P, nch], bf)
            nc.sync.dma_start(out=ldst_t[:], in_=ldst[:])

            ci = 0
            for t in range(TILES):
                nf, nb = int(sched_ft[t, 0]), int(sched_ft[t, 1])
                nt = nf + nb
                g = stage.tile([P, nt * D], bf, tag="g")
                nc.gpsimd.indirect_dma_start(
                    out=g[:], out_offset=None, in_=xt[:],
                    in_offset=bass.IndirectOffsetOnAxis(
                        ap=gidx_t[:, ci:ci + nt], axis=0),
                )
                seg = psA.tile([P, 256], f32, tag="seg")
                for typi, j0, ntyp in ((0, 0, nf), (1, nf, nb)):
                    for k in range(ntyp):
                        j = j0 + k
                        oh = ohp.tile([P, P], bf, tag="oh")
                        nc.vector.tensor_tensor(
                            out=oh[:],
                            in0=ldst_t[:, ci + j:ci + j + 1].to_broadcast(
                                [P, P]),
                            in1=iota[:],
                            op=mybir.AluOpType.is_equal,
                        )
                        nc.tensor.matmul(
                            out=seg[:, typi * P:(typi + 1) * P],
                            lhsT=g[:, j * D:(j + 1) * D], rhs=oh[:],
                            start=(k == 0), stop=(k == ntyp - 1),
                        )
                ci += nt

                accT = accp.tile([P, 2 * P], bf, tag="accT")
                nc.scalar.copy(out=accT[:], in_=seg[:, 0:2 * P])
                xs = outp.tile([P, P], bf, tag="xs")
                nc.sync.dma_start(out=xs[:], in_=xst[:, t * P:(t + 1) * P])

                gem = psB.tile([P, 256], f32, tag="gem")
                nc.tensor.matmul(out=gem[:, 0:P], lhsT=wf_t[:],
                                 rhs=accT[:, 0:P], start=True, stop=False)
                nc.tensor.matmul(out=gem[:, 0:P], lhsT=wb_t[:],
                                 rhs=accT[:, P:2 * P], start=False, stop=True)
                nc.tensor.matmul(out=gem[:, P:2 * P], lhsT=ws_t[:],
                                 rhs=xs[:], start=True, stop=True)

                du = outp.tile([P, P], f32, tag="du")
                nc.sync.dma_start(out=du[:], in_=dut[:, t * P:(t + 1) * P])
                m = outp.tile([P, P], f32, tag="m")
                nc.vector.tensor_scalar(
                    out=m[:], in0=du[:], scalar1=KEEP_PROB,
                    scalar2=1.0 / KEEP_PROB,
                    op0=mybir.AluOpType.is_lt, op1=mybir.AluOpType.mult,
                )
                sm = outp.tile([P, P], f32, tag="sm")
                nc.vector.tensor_tensor(out=sm[:], in0=gem[:, P:2 * P],
                                        in1=m[:], op=mybir.AluOpType.mult)
                tot = outp.tile([P, P], f32, tag="tot")
                nc.vector.tensor_tensor(out=tot[:], in0=gem[:, 0:P],
                                        in1=sm[:], op=mybir.AluOpType.add)
                ot = outp.tile([P, P], f32, tag="ot")
                nc.scalar.activation(out=ot[:], in_=tot[:],
                                     func=mybir.ActivationFunctionType.Relu)
                nc.sync.dma_start(out=outT[:, t * P:(t + 1) * P], in_=ot[:])
    nc.compile()
    return nc


def prepare(inputs):
    x = np.asarray(inputs["x"], np.float32)
    W_f = np.asarray(inputs["W_f"], np.float32)
    W_b = np.asarray(inputs["W_b"], np.float32)
    W_s = np.asarray(inputs["W_s"], np.float32)
    drop_u = np.asarray(inputs["drop_u"], np.float32)
    senders = np.asarray(inputs["senders"])
    receivers = np.asarray(inputs["receivers"])

    sched_ft, gidx, ldst, nch = _route(senders, receivers)
    nc = _build(sched_ft, nch)

    xt = np.zeros((N + 1, D), BF16)
    xt[:N] = _to_bf16(x)
    wfb = _to_bf16(W_f)
    wbb = _to_bf16(W_b)
    wsb = _to_bf16(W_s)
    in_maps = []
    for c in range(NCORES):
        lo = c * SHARD
        du = np.zeros((SHARD_PAD, D), np.float32)
        du[:SHARD] = drop_u[lo:lo + SHARD]
        xs = np.zeros((SHARD_PAD, D), BF16)
        xs[:SHARD] = _to_bf16(x[lo:lo + SHARD])
        in_maps.append({
            "xt": xt, "wf": wfb, "wb": wbb, "ws": wsb,
            "dut": np.ascontiguousarray(du.T),
            "gidx": np.ascontiguousarray(gidx[c]),
            "ldst": np.ascontiguousarray(_to_bf16(ldst[c])),
            "xst": np.ascontiguousarray(xs.T),
        })
    return nc, in_maps


def kernel(x, W_f, W_b, W_s, drop_u, senders, receivers):
    nc, in_maps = prepare(dict(x=x, W_f=W_f, W_b=W_b, W_s=W_s,
                               drop_u=drop_u, senders=senders,
                               receivers=receivers))
    res = run_bass_kernel_spmd(nc, in_maps, core_ids=list(range(NCORES)))
    out = np.empty((N, D), np.float32)
    for c in range(NCORES):
        out[c * SHARD:(c + 1) * SHARD] = res.results[c]["outT"][:, :SHARD].T
    return out
